# revision 49
# baseline (speedup 1.0000x reference)
"""BERT-CRF Viterbi decode kernel for Trainium2 (Bass/Tile), 8-core data parallel.

Full inputs in, full outputs out. Batch B=64 sharded across 8 cores (8
sequences each). Per core, partition rows r = b*16 + c (c = chunk of L=32
steps; C=16 chunks per sequence).

Stage A (per u-group of 4 steps): one 1.57MB DMA (12KB contiguous per row),
PE transposes the six 128-wide h-chunks (fp32), fp32r batched matmul
(W^T x sT -> emissions^T [4,512] in PSUM, 1 cyc/row), bias folded into the
ACT PSUM->SBUF copy, PE fix-transposes back to [rows, 4] -> emissions tile.

Fused under stage A, per-chunk Viterbi scans in (max,+) algebra, split at
mid-chunk into two independent fused chains on DVE (fwd + reversed bwd
share one [128,2,4,4] state slot, so each wave is one 5D TT + one
reduce_max; the two chains interleave to hide DVE dependent-op latency):
  chain-1 covers steps 0..15  (fwd red1 / bwd B1 to the step-15 pseudo-end)
  chain-2 covers steps 16..31 (fwd red2 from a trans init / bwd B2)
Group processing order 0,3,4,7,1,2,5,6 feeds both chains; G/G' step tables
(GG tile) are built by Pool from the emissions as groups land.

Tail: full-chunk matrices composed from the two half-chunk results, chunk
boundary score chains (fwd sb / bwd tb, interleaved on DVE to hide
latency), one broadcast DMA each, then tags for ALL timesteps at once:
  tag_u = argmax_j( P_u[j] + e_u[j] + Q_u[j] )
with P from {sb,sbmid} + red halves and Q from {tb,tbmid} + B halves (the
bwd half is read back via a negative-stride AP so no reversal op is
needed). No backpointers, no sequential backtracking.
"""
import sys
for p in ("/opt/trn_rl_repo", "/root/.axon_site/_ro/trn_rl_repo"):
    if p not in sys.path:
        sys.path.append(p)

import numpy as np
import concourse.bass as bass
import concourse.tile as tile
from concourse import mybir
from concourse.bass_utils import run_bass_kernel_spmd

F32 = mybir.dt.float32
F32R = mybir.dt.float32r
I32 = mybir.dt.int32
AX = mybir.AxisListType
OP = mybir.AluOpType
AF = mybir.ActivationFunctionType

B, T, H, K = 64, 512, 768, 4
NCORES = 8
BC = B // NCORES          # 8 sequences per core
C, L = 16, 32             # chunks per sequence, steps per chunk
ROWS = BC * C             # 128 partition rows
HCH = H // 128            # 6 h-chunks
UG = 4                    # steps per u-group (batched matmul width 4*128=512)
NG = L // UG              # 8 groups
PORDER = [0, 3, 4, 7, 1, 2, 5, 6]
NEG = -1.0e9

_NC_CACHE = {}


def build_nc():
    nc = bass.Bass()
    sent = nc.declare_dram_parameter("sentences", [BC, T, H], F32, isOutput=False)
    Wd = nc.declare_dram_parameter("W", [K, H], F32, isOutput=False)
    identd = nc.declare_dram_parameter("identc", [128, 128], F32, isOutput=False)
    # rowconsts[128, 64]: wfirst | biascol | binit | end | ttr | tinit
    rcd = nc.declare_dram_parameter("rowconsts", [128, 64], F32, isOutput=False)
    tagsd = nc.declare_dram_parameter("tags", [BC, T], I32, isOutput=True)

    with tile.TileContext(nc) as tc:
        with tc.tile_pool(name="singles", bufs=1) as singles, \
             tc.tile_pool(name="sent_pool", bufs=4) as sent_pool, \
             tc.tile_pool(name="st_pool", bufs=3) as st_pool, \
             tc.tile_pool(name="dve_tmp", bufs=3) as dve_tmp, \
             tc.tile_pool(name="pl_tmp", bufs=4) as pl_tmp, \
             tc.tile_pool(name="ps_tr", bufs=4, space="PSUM") as ps_tr, \
             tc.tile_pool(name="ps_eT", bufs=2, space="PSUM") as ps_eT, \
             tc.tile_pool(name="ps_fix", bufs=2, space="PSUM") as ps_fix:

            # ---------- first step's sentences + constants (pipelined start) ----------
            # Group 0 is split into per-step DMAs so the first transpose can
            # start after ~400KB instead of ~1.6MB.
            pre_sg = sent_pool.tile([128, UG, H], F32, tag="sent")
            g0 = PORDER[0]
            for uu in range(UG):
                nc.sync.dma_start(pre_sg[:, uu, :], bass.AP(
                    tensor=sent[:].tensor, offset=(g0 * UG + uu) * H,
                    ap=[[T * H, BC], [L * H, C], [1, H]]))
                if uu == 0:
                    ident = singles.tile([128, 128], F32)
                    nc.sync.dma_start(ident, identd[:])
                    rc = singles.tile([128, 64], F32)
                    nc.sync.dma_start(rc, rcd[:])
                    w_raw = singles.tile([K, H], F32)
                    nc.sync.dma_start(w_raw, Wd[:])
            iw4 = rc[:, 0:4]
            biascol = rc[0:K, 4:5]
            stinit = rc[:, 8:40].rearrange("p (h a b) -> p h a b", h=2, a=4)
            binit_xi = rc[:, 24:40].rearrange("p (x i) -> p x i", x=4)
            ttr_ij = rc[:, 40:56].rearrange("p (i j) -> p i j", i=4)
            end8 = rc[0:BC, 56:60]
            ident4 = rc[0:K, 60:64]

            # ---------- PE clock warmup ----------
            # ~10 dummy transposes as soon as the identity lands: keeps the
            # PE HAM activity window busy so the 2.4GHz clock unthrottles
            # before the first real transpose (saves ~1us of cold-clock).
            for _ in range(10):
                wup = ps_fix.tile([128, UG * K], F32, tag="fq")
                nc.tensor.transpose(wup, ident[0:UG * K, :], ident[0:UG * K, 0:UG * K])

            # ---------- W^T in SBUF: wt[p = h within chunk, ch, k] ----------
            wt_sb = singles.tile([128, HCH, K], F32R)
            for ch in range(HCH):
                wt_ps = ps_fix.tile([128, UG * K], F32, tag="fq")
                nc.tensor.transpose(wt_ps[:, 0:K],
                                    w_raw[:, ch * 128:(ch + 1) * 128], ident4)
                nc.scalar.copy(wt_sb[:, ch, :], wt_ps[:, 0:K])

            # scan emissions tile (bias included), written by stage A
            emsc = singles.tile([128, L * K], F32)
            emv = emsc.rearrange("p (u j) -> p u j", u=L)

            # fused scan state: slot s holds [red_s | Brev_s] (Brev_s = B_{31-s});
            # GG slot s holds [GT_s[j,k]=e_s[k]+tr[k,j] | G'_{31-s}[i,j]=tr[i,j]+e_{31-s}[j]]
            ST = singles.tile([128, L, 2, 4, 4], F32)
            GG = singles.tile([128, L, 2, 4, 4], F32)

            # ---------- helper emitters ----------
            def emsc_copy(g):
                nc.scalar.copy(
                    emsc[:, g * UG * K:(g + 1) * UG * K], fq_tiles[g])

            def wave(s):
                # fused step: slot s-1 -> s (chain-1: s in 1..15 = fwd step s
                # + bwd step 15-s; chain-2: s in 17..31 = fwd step s + bwd
                # step 47-s), same slot in GG
                wtmp = dve_tmp.tile([128, 2, 4, 4, 4], F32, tag="wtmp")
                nc.vector.tensor_tensor(
                    wtmp,
                    ST[:, s - 1].unsqueeze(3).to_broadcast((128, 2, 4, 4, 4)),
                    GG[:, s - 1].unsqueeze(2).to_broadcast((128, 2, 4, 4, 4)),
                    OP.add)
                nc.vector.reduce_max(ST[:, s], wtmp, axis=AX.X)

            def waves(base, lo, hi):
                for w in range(lo, hi):
                    wave(base + w)

            def g_ops(g):
                # GT_u[j,k] = e_u[k] + trans[k,j] at slots 4g..4g+3 (h=0)
                nc.gpsimd.tensor_tensor(
                    GG[:, g * UG:(g + 1) * UG, 0],
                    emv[:, g * UG:(g + 1) * UG, :].unsqueeze(2)
                        .to_broadcast((128, UG, 4, 4)),
                    ttr_ij.transpose([0, 2, 1]).unsqueeze(1)
                        .to_broadcast((128, UG, 4, 4)),
                    OP.add)
                # G'_u[i,j] = trans[i,j] + e_u[j]; chain-1 slot 15-u,
                # chain-2 slot 47-u (G'_16 parks at slot 31 h=1 for tbmid)
                for uu in range(UG):
                    u = g * UG + uu
                    slot = 15 - u if u <= 15 else 47 - u
                    nc.gpsimd.tensor_tensor(
                        GG[:, slot, 1],
                        ttr_ij,
                        emv[:, u, :].unsqueeze(1).to_broadcast((128, 4, 4)),
                        OP.add)

            # scan-block per position: chain-1 (groups 0-3) and chain-2
            # (groups 4-7) advance as their group pairs land
            SCAN_BLOCK = {1: (0, 1, 5), 3: (16, 1, 5), 5: (0, 5, 16), 7: (16, 5, 16)}

            # ---------- Stage A fused with scans, one interleaved stream ----------
            sA = nc.named_scope("stageA")
            sA.__enter__()
            nc.vector.tensor_copy(ST[:, 0], stinit)
            nc.vector.tensor_copy(ST[:, 16, 0], ttr_ij)
            nc.vector.tensor_copy(ST[:, 16, 1], binit_xi)
            fq_tiles = {}
            for pos, g in enumerate(PORDER):
                if pos == 0:
                    sg = pre_sg
                else:
                    sg = sent_pool.tile([128, UG, H], F32, tag="sent")
                    nc.sync.dma_start(sg, bass.AP(
                        tensor=sent[:].tensor, offset=g * UG * H,
                        ap=[[T * H, BC], [L * H, C], [H, UG], [1, H]]))
                sT_sb = st_pool.tile([128, HCH, UG * 128], F32R, tag="sT")
                for ch in range(HCH):
                    trp = ps_tr.tile([128, UG * 128], F32, tag="trps")
                    for uu in range(UG):
                        nc.tensor.transpose(
                            trp[:, uu * 128:(uu + 1) * 128],
                            sg[:, uu, ch * 128:(ch + 1) * 128],
                            ident)
                    nc.scalar.copy(sT_sb[:, ch, :], trp)
                eT_ps = ps_eT.tile([4, UG * 128], F32, tag="eT")
                for ch in range(HCH):
                    nc.tensor.matmul(
                        eT_ps, wt_sb[:, ch, :], sT_sb[:, ch, :],
                        start=(ch == 0), stop=(ch == HCH - 1))
                # PSUM -> SBUF with bias folded in (b[k] per partition k)
                eT_sb = st_pool.tile([4, UG * 128], F32, tag="eTsb")
                nc.scalar.activation(eT_sb, eT_ps, AF.Identity, bias=biascol)
                fq = ps_fix.tile([128, UG * K], F32, tag="fq")
                for uu in range(UG):
                    nc.tensor.transpose(
                        fq[:, uu * K:(uu + 1) * K],
                        eT_sb[:, uu * 128:(uu + 1) * 128], ident4)
                fq_tiles[g] = fq
                emsc_copy(g)
                g_ops(g)
                if pos in SCAN_BLOCK:
                    base, lo, hi = SCAN_BLOCK[pos]
                    waves(base, lo, hi)
            sA.__exit__(None, None, None)

            # ---------- chunk matrices to by-b layout ----------
            sP2 = nc.named_scope("p2")
            sP2.__enter__()
            # mid1[i,m] = red1_15[i,m] + e_15[m]; Ac[i,j] =
            # max_m(mid1[i,m] + red2_15[m,j]) + e_31[j]
            mid1 = singles.tile([128, 4, 4], F32)
            nc.vector.tensor_tensor(
                mid1, ST[:, 15, 0],
                emv[:, 15, :].unsqueeze(1).to_broadcast((128, 4, 4)),
                OP.add)
            ctmp = singles.tile([128, 4, 4, 4], F32)
            nc.vector.tensor_tensor(
                ctmp,
                mid1.unsqueeze(2).to_broadcast((128, 4, 4, 4)),
                ST[:, L - 1, 0].transpose([0, 2, 1]).unsqueeze(1)
                    .to_broadcast((128, 4, 4, 4)),
                OP.add)
            Acm = singles.tile([128, 4, 4], F32)
            nc.vector.reduce_max(Acm, ctmp, axis=AX.X)
            Ac = singles.tile([128, 16], F32)
            nc.vector.tensor_tensor(
                Ac.rearrange("p (i j) -> p i j", i=4),
                Acm,
                emv[:, L - 1, :].unsqueeze(1).to_broadcast((128, 4, 4)),
                OP.add)
            abyb = singles.tile([BC, C * 16], F32)
            nc.sync.dma_start(abyb, Ac)
            abv = abyb.rearrange("p (c i j) -> p c i j", c=C, i=4)

            # ---------- boundary chains (DVE), fwd/bwd interleaved ----------
            # The two chains are independent; interleaving hides the ~250ns
            # dependent-op latency of each.
            sbt = singles.tile([BC, 2 * C * 4], F32)
            sbv = sbt[:, 0:C * 4].rearrange("p (c j) -> p c j", c=C)
            tbv = sbt[:, C * 4:2 * C * 4].rearrange("p (c j) -> p c j", c=C)
            nc.vector.memset(sbt[:, 0:4], 0.0)
            nc.vector.tensor_copy(tbv[:, C - 1, :], end8)
            sTB = nc.named_scope("tb")
            sTB.__enter__()
            for c in range(C - 1):
                cb = C - 2 - c
                p2tmp = dve_tmp.tile([BC, 4, 4], F32, tag="p2tmp")
                # tmp[j,i] = sb_c[i] + Ac_c[i,j]
                nc.vector.tensor_tensor(
                    p2tmp,
                    sbv[:, c, :].unsqueeze(1).to_broadcast((BC, 4, 4)),
                    abv[:, c].transpose([0, 2, 1]),
                    OP.add)
                ttmp = dve_tmp.tile([BC, 4, 4], F32, tag="ttmp")
                # tmp[x,j] = Ac_{cb+1}[x,j] + tb_{cb+1}[j]
                nc.vector.tensor_tensor(
                    ttmp,
                    abv[:, cb + 1],
                    tbv[:, cb + 1, :].unsqueeze(1).to_broadcast((BC, 4, 4)),
                    OP.add)
                nc.vector.reduce_max(sbv[:, c + 1, :], p2tmp, axis=AX.X)
                nc.vector.reduce_max(tbv[:, cb, :], ttmp, axis=AX.X)
            sbc = singles.tile([128, 4], F32)
            nc.sync.dma_start(sbc, sbt[:, 0:C * 4])
            tbc = singles.tile([128, 4], F32)
            nc.sync.dma_start(tbc, sbt[:, C * 4:2 * C * 4])
            sTB.__exit__(None, None, None)

            # ---------- combine: tags for all u at once ----------
            sCB = nc.named_scope("comb")
            sCB.__enter__()
            HL = L // 2
            # Q for steps 16..31: Q[16+t][j] = max_x(B2_t[x,j] + tb[x]);
            # B2_t[x,j] = ST[31-t, 1, x, j] (negative-stride read -> normal order)
            Q = singles.tile([128, L, 4], F32)
            candQ = singles.tile([128, L, 4, 4], F32)
            bq2 = bass.AP(
                tensor=ST[:].tensor, offset=(L - 1) * 32 + 16,
                ap=[[L * 32, 128], [-32, HL], [1, 4], [4, 4]])
            nc.gpsimd.tensor_tensor(
                candQ[:, HL:L],
                bq2,
                tbc.unsqueeze(1).unsqueeze(1).to_broadcast((128, HL, 4, 4)),
                OP.add)
            nc.vector.reduce_max(Q[:, HL:L], candQ[:, HL:L], axis=AX.X)
            # tbmid[y] = max_m(G'_16[y,m] + Q_16[m])  (G'_16 parked at GG[31,1])
            tbmid = singles.tile([128, 4], F32)
            tmtmp = singles.tile([128, 4, 4], F32)
            nc.vector.tensor_tensor(
                tmtmp, GG[:, L - 1, 1],
                Q[:, HL, :].unsqueeze(1).to_broadcast((128, 4, 4)), OP.add)
            nc.vector.reduce_max(tbmid, tmtmp, axis=AX.X)
            # Q for steps 0..15: B1_u[y,j] = ST[15-u, 1, y, j]
            bq1 = bass.AP(
                tensor=ST[:].tensor, offset=(HL - 1) * 32 + 16,
                ap=[[L * 32, 128], [-32, HL], [1, 4], [4, 4]])
            nc.gpsimd.tensor_tensor(
                candQ[:, 0:HL],
                bq1,
                tbmid.unsqueeze(1).unsqueeze(1).to_broadcast((128, HL, 4, 4)),
                OP.add)
            nc.vector.reduce_max(Q[:, 0:HL], candQ[:, 0:HL], axis=AX.X)

            # sbmid[m] = max_i(sb[i] + mid1[i,m])
            sbmid = singles.tile([128, 4], F32)
            smtmp = singles.tile([128, 4, 4], F32)
            nc.vector.tensor_tensor(
                smtmp, mid1.transpose([0, 2, 1]),
                sbc.unsqueeze(1).to_broadcast((128, 4, 4)), OP.add)
            nc.vector.reduce_max(sbmid, smtmp, axis=AX.X)
            # P halves
            P = singles.tile([128, L, 4], F32)
            candP = singles.tile([128, L, 4, 4], F32)
            nc.vector.tensor_tensor(
                candP[:, 0:HL],
                ST[:, 0:HL, 0].transpose([0, 1, 3, 2]),
                sbc.unsqueeze(1).unsqueeze(1).to_broadcast((128, HL, 4, 4)),
                OP.add)
            nc.vector.reduce_max(P[:, 0:HL], candP[:, 0:HL], axis=AX.X)
            nc.vector.tensor_tensor(
                candP[:, HL:L],
                ST[:, HL:L, 0].transpose([0, 1, 3, 2]),
                sbmid.unsqueeze(1).unsqueeze(1).to_broadcast((128, HL, 4, 4)),
                OP.add)
            nc.vector.reduce_max(P[:, HL:L], candP[:, HL:L], axis=AX.X)
            R = singles.tile([128, L, 4], F32)
            nc.vector.tensor_tensor(R, P, emv, OP.add)
            nc.vector.tensor_tensor(R, R, Q, OP.add)
            M = singles.tile([128, L], F32)
            nc.vector.reduce_max(M, R, axis=AX.X)
            eq = singles.tile([128, L, 4], F32)
            nc.vector.tensor_tensor(
                eq, R, M.unsqueeze(2).to_broadcast((128, L, 4)), OP.is_equal)
            nc.vector.tensor_tensor(
                eq, eq, iw4.unsqueeze(1).to_broadcast((128, L, 4)), OP.mult)
            tagf = singles.tile([128, L], F32)
            nc.vector.reduce_max(tagf, eq, axis=AX.X)
            tagi = singles.tile([128, L], I32)
            nc.vector.tensor_copy(tagi, tagf)
            nc.sync.dma_start(tagsd[:].rearrange("b (c t) -> b c t", c=C), tagi)
            sCB.__exit__(None, None, None)

    return nc


def _split_multi_waits(nc):
    """Walrus (bass2jax path) allows very few embedded sync waits per
    instruction (PE matmul: exactly 1). Hoist multi-waits onto standalone
    single-wait InstDrain instructions on the same engine, preserving order."""
    for f in nc.m.functions:
        for blk in f.blocks:
            insts = blk.instructions
            i = 0
            while i < len(insts):
                ins = insts[i]
                si = ins.sync_info
                w = list(si.on_wait) if (si is not None and si.on_wait) else []
                if len(w) >= 2:
                    for k, wait in enumerate(w):
                        d = mybir.InstEventSemaphore(
                            name=nc.get_next_instruction_name(), ins=[], outs=[])
                        d.engine = ins.engine
                        d.sync_info = mybir.SyncInfo(on_wait=[wait], on_update=[])
                        insts.insert(i + k, d)
                    i += len(w)
                    ins.sync_info = mybir.SyncInfo(
                        on_wait=[], on_update=list(si.on_update or []))
                i += 1


def _get_nc():
    if "nc" not in _NC_CACHE:
        nc = build_nc()
        _split_multi_waits(nc)   # HW path only; CoreSim rejects raw drains
        _NC_CACHE["nc"] = nc
    return _NC_CACHE["nc"]


def make_in_maps(inputs):
    sent = np.ascontiguousarray(np.asarray(inputs["sentences"], dtype=np.float32))
    W = np.ascontiguousarray(np.asarray(inputs["W"], dtype=np.float32))
    bb = np.ascontiguousarray(np.asarray(inputs["b"], dtype=np.float32))
    st = np.ascontiguousarray(np.asarray(inputs["start_transitions"], dtype=np.float32))
    en = np.ascontiguousarray(np.asarray(inputs["end_transitions"], dtype=np.float32))
    tr = np.ascontiguousarray(np.asarray(inputs["transitions"], dtype=np.float32))
    tinit = np.tile(tr.ravel(), (128, 1)).astype(np.float32)
    tinit[0::C, :] = np.tile(st, 4)[None, :]
    binit = np.full((4, 4), NEG, dtype=np.float32)
    np.fill_diagonal(binit, 0.0)
    rc = np.zeros((128, 64), dtype=np.float32)
    rc[:, 0:4] = [0.0, 1.0, 2.0, 3.0]
    rc[0:K, 4] = bb
    rc[:, 8:24] = tinit
    rc[:, 24:40] = binit.ravel()[None, :]
    rc[:, 40:56] = tr.ravel()[None, :]
    rc[:, 56:60] = en[None, :]
    rc[0:K, 60:64] = np.eye(K, dtype=np.float32)
    identc = np.eye(128, dtype=np.float32)
    return [{
        "sentences": sent[c * BC:(c + 1) * BC],
        "W": W, "identc": identc, "rowconsts": rc,
    } for c in range(NCORES)]


def kernel(**inputs):
    nc = _get_nc()
    in_maps = make_in_maps(inputs)
    res = run_bass_kernel_spmd(nc, in_maps, core_ids=list(range(NCORES)))
    tags = np.concatenate([res.results[c]["tags"] for c in range(NCORES)], axis=0)
    return tags.astype(np.int32)


if __name__ == "__main__":
    import reference
    inputs = {k: np.asarray(v) for k, v in reference.setup_inputs().items()}
    out = kernel(**inputs)
    print(out.shape, out.dtype, out[:2, :16])


# revision 50
# speedup vs baseline: 1.0121x; 1.0121x over previous
"""BERT-CRF Viterbi decode kernel for Trainium2 (Bass/Tile), 8-core data parallel.

Full inputs in, full outputs out. Batch B=64 sharded across 8 cores (8
sequences each). Per core, partition rows r = b*16 + c (c = chunk of L=32
steps; C=16 chunks per sequence).

Stage A (per u-group of 4 steps): one 1.57MB DMA (12KB contiguous per row),
PE transposes the six 128-wide h-chunks (fp32), fp32r batched matmul
(W^T x sT -> emissions^T [4,512] in PSUM, 1 cyc/row), bias folded into the
ACT PSUM->SBUF copy, PE fix-transposes back to [rows, 4] -> emissions tile.

Fused under stage A, per-chunk Viterbi scans in (max,+) algebra, split at
mid-chunk into two independent fused chains on DVE (fwd + reversed bwd
share one [128,2,4,4] state slot, so each wave is one 5D TT + one
reduce_max; the two chains interleave to hide DVE dependent-op latency):
  chain-1 covers steps 0..15  (fwd red1 / bwd B1 to the step-15 pseudo-end)
  chain-2 covers steps 16..31 (fwd red2 from a trans init / bwd B2)
Group processing order 0,3,4,7,1,2,5,6 feeds both chains; G/G' step tables
(GG tile) are built by Pool from the emissions as groups land.

Tail: full-chunk matrices composed from the two half-chunk results, chunk
boundary score chains (fwd sb / bwd tb, interleaved on DVE to hide
latency), one broadcast DMA each, then tags for ALL timesteps at once:
  tag_u = argmax_j( P_u[j] + e_u[j] + Q_u[j] )
with P from {sb,sbmid} + red halves and Q from {tb,tbmid} + B halves (the
bwd half is read back via a negative-stride AP so no reversal op is
needed). No backpointers, no sequential backtracking.
"""
import sys
for p in ("/opt/trn_rl_repo", "/root/.axon_site/_ro/trn_rl_repo"):
    if p not in sys.path:
        sys.path.append(p)

import numpy as np
import concourse.bass as bass
import concourse.tile as tile
from concourse import mybir
from concourse.bass_utils import run_bass_kernel_spmd

F32 = mybir.dt.float32
F32R = mybir.dt.float32r
I32 = mybir.dt.int32
AX = mybir.AxisListType
OP = mybir.AluOpType
AF = mybir.ActivationFunctionType

B, T, H, K = 64, 512, 768, 4
NCORES = 8
BC = B // NCORES          # 8 sequences per core
C, L = 16, 32             # chunks per sequence, steps per chunk
ROWS = BC * C             # 128 partition rows
HCH = H // 128            # 6 h-chunks
UG = 4                    # steps per u-group (batched matmul width 4*128=512)
NG = L // UG              # 8 groups
PORDER = [0, 3, 4, 7, 1, 2, 5, 6]
NEG = -1.0e9

_NC_CACHE = {}


def build_nc():
    nc = bass.Bass()
    sent = nc.declare_dram_parameter("sentences", [BC, T, H], F32, isOutput=False)
    Wd = nc.declare_dram_parameter("W", [K, H], F32, isOutput=False)
    identd = nc.declare_dram_parameter("identc", [128, 128], F32, isOutput=False)
    # rowconsts[128, 64]: wfirst | biascol | binit | end | ttr | tinit
    rcd = nc.declare_dram_parameter("rowconsts", [128, 64], F32, isOutput=False)
    tagsd = nc.declare_dram_parameter("tags", [BC, T], I32, isOutput=True)

    with tile.TileContext(nc) as tc:
        with tc.tile_pool(name="singles", bufs=1) as singles, \
             tc.tile_pool(name="sent_pool", bufs=4) as sent_pool, \
             tc.tile_pool(name="st_pool", bufs=3) as st_pool, \
             tc.tile_pool(name="dve_tmp", bufs=3) as dve_tmp, \
             tc.tile_pool(name="pl_tmp", bufs=4) as pl_tmp, \
             tc.tile_pool(name="ps_tr", bufs=4, space="PSUM") as ps_tr, \
             tc.tile_pool(name="ps_eT", bufs=2, space="PSUM") as ps_eT, \
             tc.tile_pool(name="ps_fix", bufs=2, space="PSUM") as ps_fix:

            # ---------- first step's sentences + constants (pipelined start) ----------
            # Group 0 is split into per-step DMAs so the first transpose can
            # start after ~400KB instead of ~1.6MB.
            pre_sg = sent_pool.tile([128, UG, H], F32, tag="sent")
            g0 = PORDER[0]
            for uu in range(UG):
                nc.sync.dma_start(pre_sg[:, uu, :], bass.AP(
                    tensor=sent[:].tensor, offset=(g0 * UG + uu) * H,
                    ap=[[T * H, BC], [L * H, C], [1, H]]))
                if uu == 0:
                    ident = singles.tile([128, 128], F32)
                    nc.sync.dma_start(ident, identd[:])
                    rc = singles.tile([128, 64], F32)
                    nc.sync.dma_start(rc, rcd[:])
                    w_raw = singles.tile([K, H], F32)
                    nc.sync.dma_start(w_raw, Wd[:])
            iw4 = rc[:, 0:4]
            biascol = rc[0:K, 4:5]
            stinit = rc[:, 8:40].rearrange("p (h a b) -> p h a b", h=2, a=4)
            binit_xi = rc[:, 24:40].rearrange("p (x i) -> p x i", x=4)
            ttr_ij = rc[:, 40:56].rearrange("p (i j) -> p i j", i=4)
            end8 = rc[0:BC, 56:60]
            ident4 = rc[0:K, 60:64]

            # ---------- W^T in SBUF: wt[p = h within chunk, ch, k] ----------
            wt_sb = singles.tile([128, HCH, K], F32R)
            for ch in range(HCH):
                wt_ps = ps_fix.tile([128, UG * K], F32, tag="fq")
                nc.tensor.transpose(wt_ps[:, 0:K],
                                    w_raw[:, ch * 128:(ch + 1) * 128], ident4)
                nc.scalar.copy(wt_sb[:, ch, :], wt_ps[:, 0:K])

            # scan emissions tile (bias included), written by stage A
            emsc = singles.tile([128, L * K], F32)
            emv = emsc.rearrange("p (u j) -> p u j", u=L)

            # fused scan state: slot s holds [red_s | Brev_s] (Brev_s = B_{31-s});
            # GG slot s holds [GT_s[j,k]=e_s[k]+tr[k,j] | G'_{31-s}[i,j]=tr[i,j]+e_{31-s}[j]]
            ST = singles.tile([128, L, 2, 4, 4], F32)
            GG = singles.tile([128, L, 2, 4, 4], F32)

            # ---------- helper emitters ----------
            def emsc_copy(g):
                nc.scalar.copy(
                    emsc[:, g * UG * K:(g + 1) * UG * K], fq_tiles[g])

            def wave(s):
                # fused step: slot s-1 -> s (chain-1: s in 1..15 = fwd step s
                # + bwd step 15-s; chain-2: s in 17..31 = fwd step s + bwd
                # step 47-s), same slot in GG
                wtmp = dve_tmp.tile([128, 2, 4, 4, 4], F32, tag="wtmp")
                nc.vector.tensor_tensor(
                    wtmp,
                    ST[:, s - 1].unsqueeze(3).to_broadcast((128, 2, 4, 4, 4)),
                    GG[:, s - 1].unsqueeze(2).to_broadcast((128, 2, 4, 4, 4)),
                    OP.add)
                nc.vector.reduce_max(ST[:, s], wtmp, axis=AX.X)

            def waves(base, lo, hi):
                for w in range(lo, hi):
                    wave(base + w)

            def g_ops(g):
                # GT_u[j,k] = e_u[k] + trans[k,j] at slots 4g..4g+3 (h=0)
                nc.gpsimd.tensor_tensor(
                    GG[:, g * UG:(g + 1) * UG, 0],
                    emv[:, g * UG:(g + 1) * UG, :].unsqueeze(2)
                        .to_broadcast((128, UG, 4, 4)),
                    ttr_ij.transpose([0, 2, 1]).unsqueeze(1)
                        .to_broadcast((128, UG, 4, 4)),
                    OP.add)
                # G'_u[i,j] = trans[i,j] + e_u[j]; chain-1 slot 15-u,
                # chain-2 slot 47-u (G'_16 parks at slot 31 h=1 for tbmid)
                for uu in range(UG):
                    u = g * UG + uu
                    slot = 15 - u if u <= 15 else 47 - u
                    nc.gpsimd.tensor_tensor(
                        GG[:, slot, 1],
                        ttr_ij,
                        emv[:, u, :].unsqueeze(1).to_broadcast((128, 4, 4)),
                        OP.add)

            # scan-block per position: chain-1 (groups 0-3) and chain-2
            # (groups 4-7) advance as their group pairs land
            SCAN_BLOCK = {1: (0, 1, 5), 3: (16, 1, 5), 5: (0, 5, 16), 7: (16, 5, 16)}

            # ---------- Stage A fused with scans, one interleaved stream ----------
            sA = nc.named_scope("stageA")
            sA.__enter__()
            nc.vector.tensor_copy(ST[:, 0], stinit)
            nc.vector.tensor_copy(ST[:, 16, 0], ttr_ij)
            nc.vector.tensor_copy(ST[:, 16, 1], binit_xi)
            fq_tiles = {}
            for pos, g in enumerate(PORDER):
                if pos == 0:
                    sg = pre_sg
                else:
                    sg = sent_pool.tile([128, UG, H], F32, tag="sent")
                    nc.sync.dma_start(sg, bass.AP(
                        tensor=sent[:].tensor, offset=g * UG * H,
                        ap=[[T * H, BC], [L * H, C], [H, UG], [1, H]]))
                sT_sb = st_pool.tile([128, HCH, UG * 128], F32R, tag="sT")
                for ch in range(HCH):
                    trp = ps_tr.tile([128, UG * 128], F32, tag="trps")
                    for uu in range(UG):
                        nc.tensor.transpose(
                            trp[:, uu * 128:(uu + 1) * 128],
                            sg[:, uu, ch * 128:(ch + 1) * 128],
                            ident)
                    nc.scalar.copy(sT_sb[:, ch, :], trp)
                eT_ps = ps_eT.tile([4, UG * 128], F32, tag="eT")
                for ch in range(HCH):
                    nc.tensor.matmul(
                        eT_ps, wt_sb[:, ch, :], sT_sb[:, ch, :],
                        start=(ch == 0), stop=(ch == HCH - 1))
                # PSUM -> SBUF with bias folded in (b[k] per partition k)
                eT_sb = st_pool.tile([4, UG * 128], F32, tag="eTsb")
                nc.scalar.activation(eT_sb, eT_ps, AF.Identity, bias=biascol)
                fq = ps_fix.tile([128, UG * K], F32, tag="fq")
                for uu in range(UG):
                    nc.tensor.transpose(
                        fq[:, uu * K:(uu + 1) * K],
                        eT_sb[:, uu * 128:(uu + 1) * 128], ident4)
                fq_tiles[g] = fq
                emsc_copy(g)
                g_ops(g)
                if pos in SCAN_BLOCK:
                    base, lo, hi = SCAN_BLOCK[pos]
                    waves(base, lo, hi)
            sA.__exit__(None, None, None)

            # ---------- chunk matrices to by-b layout ----------
            sP2 = nc.named_scope("p2")
            sP2.__enter__()
            # mid1[i,m] = red1_15[i,m] + e_15[m]; Ac[i,j] =
            # max_m(mid1[i,m] + red2_15[m,j]) + e_31[j]
            mid1 = singles.tile([128, 4, 4], F32)
            nc.vector.tensor_tensor(
                mid1, ST[:, 15, 0],
                emv[:, 15, :].unsqueeze(1).to_broadcast((128, 4, 4)),
                OP.add)
            ctmp = singles.tile([128, 4, 4, 4], F32)
            nc.vector.tensor_tensor(
                ctmp,
                mid1.unsqueeze(2).to_broadcast((128, 4, 4, 4)),
                ST[:, L - 1, 0].transpose([0, 2, 1]).unsqueeze(1)
                    .to_broadcast((128, 4, 4, 4)),
                OP.add)
            Acm = singles.tile([128, 4, 4], F32)
            nc.vector.reduce_max(Acm, ctmp, axis=AX.X)
            Ac = singles.tile([128, 16], F32)
            nc.vector.tensor_tensor(
                Ac.rearrange("p (i j) -> p i j", i=4),
                Acm,
                emv[:, L - 1, :].unsqueeze(1).to_broadcast((128, 4, 4)),
                OP.add)
            abyb = singles.tile([BC, C * 16], F32)
            nc.sync.dma_start(abyb, Ac)
            abv = abyb.rearrange("p (c i j) -> p c i j", c=C, i=4)

            # ---------- boundary chains (DVE), fwd/bwd interleaved ----------
            # The two chains are independent; interleaving hides the ~250ns
            # dependent-op latency of each.
            sbt = singles.tile([BC, 2 * C * 4], F32)
            sbv = sbt[:, 0:C * 4].rearrange("p (c j) -> p c j", c=C)
            tbv = sbt[:, C * 4:2 * C * 4].rearrange("p (c j) -> p c j", c=C)
            nc.vector.memset(sbt[:, 0:4], 0.0)
            nc.vector.tensor_copy(tbv[:, C - 1, :], end8)
            sTB = nc.named_scope("tb")
            sTB.__enter__()
            for c in range(C - 1):
                cb = C - 2 - c
                p2tmp = dve_tmp.tile([BC, 4, 4], F32, tag="p2tmp")
                # tmp[j,i] = sb_c[i] + Ac_c[i,j]
                nc.vector.tensor_tensor(
                    p2tmp,
                    sbv[:, c, :].unsqueeze(1).to_broadcast((BC, 4, 4)),
                    abv[:, c].transpose([0, 2, 1]),
                    OP.add)
                ttmp = dve_tmp.tile([BC, 4, 4], F32, tag="ttmp")
                # tmp[x,j] = Ac_{cb+1}[x,j] + tb_{cb+1}[j]
                nc.vector.tensor_tensor(
                    ttmp,
                    abv[:, cb + 1],
                    tbv[:, cb + 1, :].unsqueeze(1).to_broadcast((BC, 4, 4)),
                    OP.add)
                nc.vector.reduce_max(sbv[:, c + 1, :], p2tmp, axis=AX.X)
                nc.vector.reduce_max(tbv[:, cb, :], ttmp, axis=AX.X)
            sbc = singles.tile([128, 4], F32)
            nc.sync.dma_start(sbc, sbt[:, 0:C * 4])
            tbc = singles.tile([128, 4], F32)
            nc.sync.dma_start(tbc, sbt[:, C * 4:2 * C * 4])
            sTB.__exit__(None, None, None)

            # ---------- combine: tags for all u at once ----------
            sCB = nc.named_scope("comb")
            sCB.__enter__()
            HL = L // 2
            # Q for steps 16..31: Q[16+t][j] = max_x(B2_t[x,j] + tb[x]);
            # B2_t[x,j] = ST[31-t, 1, x, j] (negative-stride read -> normal order)
            Q = singles.tile([128, L, 4], F32)
            candQ = singles.tile([128, L, 4, 4], F32)
            bq2 = bass.AP(
                tensor=ST[:].tensor, offset=(L - 1) * 32 + 16,
                ap=[[L * 32, 128], [-32, HL], [1, 4], [4, 4]])
            nc.gpsimd.tensor_tensor(
                candQ[:, HL:L],
                bq2,
                tbc.unsqueeze(1).unsqueeze(1).to_broadcast((128, HL, 4, 4)),
                OP.add)
            nc.vector.reduce_max(Q[:, HL:L], candQ[:, HL:L], axis=AX.X)
            # tbmid[y] = max_m(G'_16[y,m] + Q_16[m])  (G'_16 parked at GG[31,1])
            tbmid = singles.tile([128, 4], F32)
            tmtmp = singles.tile([128, 4, 4], F32)
            nc.vector.tensor_tensor(
                tmtmp, GG[:, L - 1, 1],
                Q[:, HL, :].unsqueeze(1).to_broadcast((128, 4, 4)), OP.add)
            nc.vector.reduce_max(tbmid, tmtmp, axis=AX.X)
            # Q for steps 0..15: B1_u[y,j] = ST[15-u, 1, y, j]
            bq1 = bass.AP(
                tensor=ST[:].tensor, offset=(HL - 1) * 32 + 16,
                ap=[[L * 32, 128], [-32, HL], [1, 4], [4, 4]])
            nc.gpsimd.tensor_tensor(
                candQ[:, 0:HL],
                bq1,
                tbmid.unsqueeze(1).unsqueeze(1).to_broadcast((128, HL, 4, 4)),
                OP.add)
            nc.vector.reduce_max(Q[:, 0:HL], candQ[:, 0:HL], axis=AX.X)

            # sbmid[m] = max_i(sb[i] + mid1[i,m])
            sbmid = singles.tile([128, 4], F32)
            smtmp = singles.tile([128, 4, 4], F32)
            nc.vector.tensor_tensor(
                smtmp, mid1.transpose([0, 2, 1]),
                sbc.unsqueeze(1).to_broadcast((128, 4, 4)), OP.add)
            nc.vector.reduce_max(sbmid, smtmp, axis=AX.X)
            # P halves
            P = singles.tile([128, L, 4], F32)
            candP = singles.tile([128, L, 4, 4], F32)
            nc.vector.tensor_tensor(
                candP[:, 0:HL],
                ST[:, 0:HL, 0].transpose([0, 1, 3, 2]),
                sbc.unsqueeze(1).unsqueeze(1).to_broadcast((128, HL, 4, 4)),
                OP.add)
            nc.vector.reduce_max(P[:, 0:HL], candP[:, 0:HL], axis=AX.X)
            nc.vector.tensor_tensor(
                candP[:, HL:L],
                ST[:, HL:L, 0].transpose([0, 1, 3, 2]),
                sbmid.unsqueeze(1).unsqueeze(1).to_broadcast((128, HL, 4, 4)),
                OP.add)
            nc.vector.reduce_max(P[:, HL:L], candP[:, HL:L], axis=AX.X)
            R = singles.tile([128, L, 4], F32)
            nc.vector.tensor_tensor(R, P, emv, OP.add)
            nc.vector.tensor_tensor(R, R, Q, OP.add)
            M = singles.tile([128, L], F32)
            nc.vector.reduce_max(M, R, axis=AX.X)
            eq = singles.tile([128, L, 4], F32)
            nc.vector.tensor_tensor(
                eq, R, M.unsqueeze(2).to_broadcast((128, L, 4)), OP.is_equal)
            nc.vector.tensor_tensor(
                eq, eq, iw4.unsqueeze(1).to_broadcast((128, L, 4)), OP.mult)
            tagf = singles.tile([128, L], F32)
            nc.vector.reduce_max(tagf, eq, axis=AX.X)
            tagi = singles.tile([128, L], I32)
            nc.vector.tensor_copy(tagi, tagf)
            nc.sync.dma_start(tagsd[:].rearrange("b (c t) -> b c t", c=C), tagi)
            sCB.__exit__(None, None, None)

    return nc


def _split_multi_waits(nc):
    """Walrus (bass2jax path) allows very few embedded sync waits per
    instruction (PE matmul: exactly 1). Hoist multi-waits onto standalone
    single-wait InstDrain instructions on the same engine, preserving order."""
    for f in nc.m.functions:
        for blk in f.blocks:
            insts = blk.instructions
            i = 0
            while i < len(insts):
                ins = insts[i]
                si = ins.sync_info
                w = list(si.on_wait) if (si is not None and si.on_wait) else []
                if len(w) >= 2:
                    for k, wait in enumerate(w):
                        d = mybir.InstEventSemaphore(
                            name=nc.get_next_instruction_name(), ins=[], outs=[])
                        d.engine = ins.engine
                        d.sync_info = mybir.SyncInfo(on_wait=[wait], on_update=[])
                        insts.insert(i + k, d)
                    i += len(w)
                    ins.sync_info = mybir.SyncInfo(
                        on_wait=[], on_update=list(si.on_update or []))
                i += 1


def _get_nc():
    if "nc" not in _NC_CACHE:
        nc = build_nc()
        _split_multi_waits(nc)   # HW path only; CoreSim rejects raw drains
        _NC_CACHE["nc"] = nc
    return _NC_CACHE["nc"]


def make_in_maps(inputs):
    sent = np.ascontiguousarray(np.asarray(inputs["sentences"], dtype=np.float32))
    W = np.ascontiguousarray(np.asarray(inputs["W"], dtype=np.float32))
    bb = np.ascontiguousarray(np.asarray(inputs["b"], dtype=np.float32))
    st = np.ascontiguousarray(np.asarray(inputs["start_transitions"], dtype=np.float32))
    en = np.ascontiguousarray(np.asarray(inputs["end_transitions"], dtype=np.float32))
    tr = np.ascontiguousarray(np.asarray(inputs["transitions"], dtype=np.float32))
    tinit = np.tile(tr.ravel(), (128, 1)).astype(np.float32)
    tinit[0::C, :] = np.tile(st, 4)[None, :]
    binit = np.full((4, 4), NEG, dtype=np.float32)
    np.fill_diagonal(binit, 0.0)
    rc = np.zeros((128, 64), dtype=np.float32)
    rc[:, 0:4] = [0.0, 1.0, 2.0, 3.0]
    rc[0:K, 4] = bb
    rc[:, 8:24] = tinit
    rc[:, 24:40] = binit.ravel()[None, :]
    rc[:, 40:56] = tr.ravel()[None, :]
    rc[:, 56:60] = en[None, :]
    rc[0:K, 60:64] = np.eye(K, dtype=np.float32)
    identc = np.eye(128, dtype=np.float32)
    return [{
        "sentences": sent[c * BC:(c + 1) * BC],
        "W": W, "identc": identc, "rowconsts": rc,
    } for c in range(NCORES)]


def kernel(**inputs):
    nc = _get_nc()
    in_maps = make_in_maps(inputs)
    res = run_bass_kernel_spmd(nc, in_maps, core_ids=list(range(NCORES)))
    tags = np.concatenate([res.results[c]["tags"] for c in range(NCORES)], axis=0)
    return tags.astype(np.int32)


if __name__ == "__main__":
    import reference
    inputs = {k: np.asarray(v) for k, v in reference.setup_inputs().items()}
    out = kernel(**inputs)
    print(out.shape, out.dtype, out[:2, :16])


# revision 51
# speedup vs baseline: 1.0140x; 1.0019x over previous
"""BERT-CRF Viterbi decode kernel for Trainium2 (Bass/Tile), 8-core data parallel.

Full inputs in, full outputs out. Batch B=64 sharded across 8 cores (8
sequences each). Per core, partition rows r = b*16 + c (c = chunk of L=32
steps; C=16 chunks per sequence).

Stage A (per u-group of 4 steps): one 1.57MB DMA (12KB contiguous per row),
PE transposes the six 128-wide h-chunks (fp32), fp32r batched matmul
(W^T x sT -> emissions^T [4,512] in PSUM, 1 cyc/row), bias folded into the
ACT PSUM->SBUF copy, PE fix-transposes back to [rows, 4] -> emissions tile.

Fused under stage A, per-chunk Viterbi scans in (max,+) algebra, split at
mid-chunk into two independent fused chains on DVE (fwd + reversed bwd
share one [128,2,4,4] state slot, so each wave is one 5D TT + one
reduce_max; the two chains interleave to hide DVE dependent-op latency):
  chain-1 covers steps 0..15  (fwd red1 / bwd B1 to the step-15 pseudo-end)
  chain-2 covers steps 16..31 (fwd red2 from a trans init / bwd B2)
Group processing order 0,3,4,7,1,2,5,6 feeds both chains; G/G' step tables
(GG tile) are built by Pool from the emissions as groups land.

Tail: full-chunk matrices composed from the two half-chunk results, chunk
boundary score chains (fwd sb / bwd tb, interleaved on DVE to hide
latency), one broadcast DMA each, then tags for ALL timesteps at once:
  tag_u = argmax_j( P_u[j] + e_u[j] + Q_u[j] )
with P from {sb,sbmid} + red halves and Q from {tb,tbmid} + B halves (the
bwd half is read back via a negative-stride AP so no reversal op is
needed). No backpointers, no sequential backtracking.
"""
import sys
for p in ("/opt/trn_rl_repo", "/root/.axon_site/_ro/trn_rl_repo"):
    if p not in sys.path:
        sys.path.append(p)

import numpy as np
import concourse.bass as bass
import concourse.tile as tile
from concourse import mybir
from concourse.bass_utils import run_bass_kernel_spmd

F32 = mybir.dt.float32
F32R = mybir.dt.float32r
I32 = mybir.dt.int32
AX = mybir.AxisListType
OP = mybir.AluOpType
AF = mybir.ActivationFunctionType

B, T, H, K = 64, 512, 768, 4
NCORES = 8
BC = B // NCORES          # 8 sequences per core
C, L = 16, 32             # chunks per sequence, steps per chunk
ROWS = BC * C             # 128 partition rows
HCH = H // 128            # 6 h-chunks
UG = 4                    # steps per u-group (batched matmul width 4*128=512)
NG = L // UG              # 8 groups
PORDER = [0, 3, 4, 7, 1, 2, 5, 6]
NEG = -1.0e9

_NC_CACHE = {}


def build_nc():
    nc = bass.Bass()
    sent = nc.declare_dram_parameter("sentences", [BC, T, H], F32, isOutput=False)
    Wd = nc.declare_dram_parameter("W", [K, H], F32, isOutput=False)
    identd = nc.declare_dram_parameter("identc", [128, 128], F32, isOutput=False)
    # rowconsts[128, 64]: wfirst | biascol | binit | end | ttr | tinit
    rcd = nc.declare_dram_parameter("rowconsts", [128, 64], F32, isOutput=False)
    tagsd = nc.declare_dram_parameter("tags", [BC, T], I32, isOutput=True)

    with tile.TileContext(nc) as tc:
        with tc.tile_pool(name="singles", bufs=1) as singles, \
             tc.tile_pool(name="sent_pool", bufs=4) as sent_pool, \
             tc.tile_pool(name="st_pool", bufs=3) as st_pool, \
             tc.tile_pool(name="dve_tmp", bufs=3) as dve_tmp, \
             tc.tile_pool(name="pl_tmp", bufs=4) as pl_tmp, \
             tc.tile_pool(name="ps_tr", bufs=4, space="PSUM") as ps_tr, \
             tc.tile_pool(name="ps_eT", bufs=2, space="PSUM") as ps_eT, \
             tc.tile_pool(name="ps_fix", bufs=2, space="PSUM") as ps_fix:

            # ---------- first step's sentences + constants (pipelined start) ----------
            # Group 0 is split into per-step DMAs so the first transpose can
            # start after ~400KB instead of ~1.6MB.
            pre_sg = sent_pool.tile([128, UG, H], F32, tag="sent")
            g0 = PORDER[0]
            for uu in range(UG):
                nc.sync.dma_start(pre_sg[:, uu, :], bass.AP(
                    tensor=sent[:].tensor, offset=(g0 * UG + uu) * H,
                    ap=[[T * H, BC], [L * H, C], [1, H]]))
                if uu == 0:
                    ident = singles.tile([128, 128], F32)
                    nc.sync.dma_start(ident, identd[:])
                    rc = singles.tile([128, 64], F32)
                    nc.sync.dma_start(rc, rcd[:])
                    w_raw = singles.tile([K, H], F32)
                    nc.sync.dma_start(w_raw, Wd[:])
            iw4 = rc[:, 0:4]
            biascol = rc[0:K, 4:5]
            stinit = rc[:, 8:40].rearrange("p (h a b) -> p h a b", h=2, a=4)
            binit_xi = rc[:, 24:40].rearrange("p (x i) -> p x i", x=4)
            ttr_ij = rc[:, 40:56].rearrange("p (i j) -> p i j", i=4)
            end8 = rc[0:BC, 56:60]
            ident4 = rc[0:K, 60:64]

            # ---------- W^T in SBUF: wt[p = h within chunk, ch, k] ----------
            wt_sb = singles.tile([128, HCH, K], F32R)
            for ch in range(HCH):
                wt_ps = ps_fix.tile([128, UG * K], F32, tag="fq")
                nc.tensor.transpose(wt_ps[:, 0:K],
                                    w_raw[:, ch * 128:(ch + 1) * 128], ident4)
                nc.scalar.copy(wt_sb[:, ch, :], wt_ps[:, 0:K])

            # scan emissions tile (bias included), written by stage A
            emsc = singles.tile([128, L * K], F32)
            emv = emsc.rearrange("p (u j) -> p u j", u=L)

            # fused scan state: slot s holds [red_s | Brev_s] (Brev_s = B_{31-s});
            # GG slot s holds [GT_s[j,k]=e_s[k]+tr[k,j] | G'_{31-s}[i,j]=tr[i,j]+e_{31-s}[j]]
            ST = singles.tile([128, L, 2, 4, 4], F32)
            GG = singles.tile([128, L, 2, 4, 4], F32)

            # ---------- helper emitters ----------
            def emsc_copy(g):
                nc.scalar.copy(
                    emsc[:, g * UG * K:(g + 1) * UG * K], fq_tiles[g])

            def wave(s):
                # fused step: slot s-1 -> s (chain-1: s in 1..15 = fwd step s
                # + bwd step 15-s; chain-2: s in 17..31 = fwd step s + bwd
                # step 47-s), same slot in GG
                wtmp = dve_tmp.tile([128, 2, 4, 4, 4], F32, tag="wtmp")
                nc.vector.tensor_tensor(
                    wtmp,
                    ST[:, s - 1].unsqueeze(3).to_broadcast((128, 2, 4, 4, 4)),
                    GG[:, s - 1].unsqueeze(2).to_broadcast((128, 2, 4, 4, 4)),
                    OP.add)
                nc.vector.reduce_max(ST[:, s], wtmp, axis=AX.X)

            def waves(base, lo, hi):
                for w in range(lo, hi):
                    wave(base + w)

            def g_ops(g):
                # GT_u[j,k] = e_u[k] + trans[k,j] at slots 4g..4g+3 (h=0)
                nc.gpsimd.tensor_tensor(
                    GG[:, g * UG:(g + 1) * UG, 0],
                    emv[:, g * UG:(g + 1) * UG, :].unsqueeze(2)
                        .to_broadcast((128, UG, 4, 4)),
                    ttr_ij.transpose([0, 2, 1]).unsqueeze(1)
                        .to_broadcast((128, UG, 4, 4)),
                    OP.add)
                # G'_u[i,j] = trans[i,j] + e_u[j]; chain-1 slot 15-u,
                # chain-2 slot 47-u (G'_16 parks at slot 31 h=1 for tbmid)
                for uu in range(UG):
                    u = g * UG + uu
                    slot = 15 - u if u <= 15 else 47 - u
                    nc.gpsimd.tensor_tensor(
                        GG[:, slot, 1],
                        ttr_ij,
                        emv[:, u, :].unsqueeze(1).to_broadcast((128, 4, 4)),
                        OP.add)

            # scan-block per position: chain-1 (groups 0-3) and chain-2
            # (groups 4-7) advance as their group pairs land
            SCAN_BLOCK = {1: (0, 1, 5), 3: (16, 1, 5), 5: (0, 5, 16), 7: (16, 5, 16)}
            mid1 = singles.tile([128, 4, 4], F32)

            # ---------- Stage A fused with scans, one interleaved stream ----------
            sA = nc.named_scope("stageA")
            sA.__enter__()
            nc.vector.tensor_copy(ST[:, 0], stinit)
            nc.vector.tensor_copy(ST[:, 16, 0], ttr_ij)
            nc.vector.tensor_copy(ST[:, 16, 1], binit_xi)
            fq_tiles = {}
            for pos, g in enumerate(PORDER):
                if pos == 0:
                    sg = pre_sg
                else:
                    sg = sent_pool.tile([128, UG, H], F32, tag="sent")
                    nc.sync.dma_start(sg, bass.AP(
                        tensor=sent[:].tensor, offset=g * UG * H,
                        ap=[[T * H, BC], [L * H, C], [H, UG], [1, H]]))
                sT_sb = st_pool.tile([128, HCH, UG * 128], F32R, tag="sT")
                for ch in range(HCH):
                    trp = ps_tr.tile([128, UG * 128], F32, tag="trps")
                    for uu in range(UG):
                        nc.tensor.transpose(
                            trp[:, uu * 128:(uu + 1) * 128],
                            sg[:, uu, ch * 128:(ch + 1) * 128],
                            ident)
                    nc.scalar.copy(sT_sb[:, ch, :], trp)
                eT_ps = ps_eT.tile([4, UG * 128], F32, tag="eT")
                for ch in range(HCH):
                    nc.tensor.matmul(
                        eT_ps, wt_sb[:, ch, :], sT_sb[:, ch, :],
                        start=(ch == 0), stop=(ch == HCH - 1))
                # PSUM -> SBUF with bias folded in (b[k] per partition k)
                eT_sb = st_pool.tile([4, UG * 128], F32, tag="eTsb")
                nc.scalar.activation(eT_sb, eT_ps, AF.Identity, bias=biascol)
                fq = ps_fix.tile([128, UG * K], F32, tag="fq")
                for uu in range(UG):
                    nc.tensor.transpose(
                        fq[:, uu * K:(uu + 1) * K],
                        eT_sb[:, uu * 128:(uu + 1) * 128], ident4)
                fq_tiles[g] = fq
                emsc_copy(g)
                g_ops(g)
                if pos in SCAN_BLOCK:
                    base, lo, hi = SCAN_BLOCK[pos]
                    waves(base, lo, hi)
                if pos == 5:
                    # mid1[i,m] = red1_15[i,m] + e_15[m] (chain-1 just ended)
                    nc.vector.tensor_tensor(
                        mid1, ST[:, 15, 0],
                        emv[:, 15, :].unsqueeze(1).to_broadcast((128, 4, 4)),
                        OP.add)
            sA.__exit__(None, None, None)

            # ---------- chunk matrices to by-b layout ----------
            sP2 = nc.named_scope("p2")
            sP2.__enter__()
            # Ac[i,j] = max_m(mid1[i,m] + red2_15[m,j]) + e_31[j]
            ctmp = singles.tile([128, 4, 4, 4], F32)
            nc.vector.tensor_tensor(
                ctmp,
                mid1.unsqueeze(2).to_broadcast((128, 4, 4, 4)),
                ST[:, L - 1, 0].transpose([0, 2, 1]).unsqueeze(1)
                    .to_broadcast((128, 4, 4, 4)),
                OP.add)
            Acm = singles.tile([128, 4, 4], F32)
            nc.vector.reduce_max(Acm, ctmp, axis=AX.X)
            Ac = singles.tile([128, 16], F32)
            nc.vector.tensor_tensor(
                Ac.rearrange("p (i j) -> p i j", i=4),
                Acm,
                emv[:, L - 1, :].unsqueeze(1).to_broadcast((128, 4, 4)),
                OP.add)
            abyb = singles.tile([BC, C * 16], F32)
            nc.sync.dma_start(abyb, Ac)
            abv = abyb.rearrange("p (c i j) -> p c i j", c=C, i=4)

            # ---------- boundary chains (DVE), fwd/bwd interleaved ----------
            # The two chains are independent; interleaving hides the ~250ns
            # dependent-op latency of each.
            sbt = singles.tile([BC, 2 * C * 4], F32)
            sbv = sbt[:, 0:C * 4].rearrange("p (c j) -> p c j", c=C)
            tbv = sbt[:, C * 4:2 * C * 4].rearrange("p (c j) -> p c j", c=C)
            nc.vector.memset(sbt[:, 0:4], 0.0)
            nc.vector.tensor_copy(tbv[:, C - 1, :], end8)
            sTB = nc.named_scope("tb")
            sTB.__enter__()
            for c in range(C - 1):
                cb = C - 2 - c
                p2tmp = dve_tmp.tile([BC, 4, 4], F32, tag="p2tmp")
                # tmp[j,i] = sb_c[i] + Ac_c[i,j]
                nc.vector.tensor_tensor(
                    p2tmp,
                    sbv[:, c, :].unsqueeze(1).to_broadcast((BC, 4, 4)),
                    abv[:, c].transpose([0, 2, 1]),
                    OP.add)
                ttmp = dve_tmp.tile([BC, 4, 4], F32, tag="ttmp")
                # tmp[x,j] = Ac_{cb+1}[x,j] + tb_{cb+1}[j]
                nc.vector.tensor_tensor(
                    ttmp,
                    abv[:, cb + 1],
                    tbv[:, cb + 1, :].unsqueeze(1).to_broadcast((BC, 4, 4)),
                    OP.add)
                nc.vector.reduce_max(sbv[:, c + 1, :], p2tmp, axis=AX.X)
                nc.vector.reduce_max(tbv[:, cb, :], ttmp, axis=AX.X)
            sbc = singles.tile([128, 4], F32)
            nc.sync.dma_start(sbc, sbt[:, 0:C * 4])
            tbc = singles.tile([128, 4], F32)
            nc.sync.dma_start(tbc, sbt[:, C * 4:2 * C * 4])
            sTB.__exit__(None, None, None)

            # ---------- combine: tags for all u at once ----------
            sCB = nc.named_scope("comb")
            sCB.__enter__()
            HL = L // 2
            # Q for steps 16..31: Q[16+t][j] = max_x(B2_t[x,j] + tb[x]);
            # B2_t[x,j] = ST[31-t, 1, x, j] (negative-stride read -> normal order)
            Q = singles.tile([128, L, 4], F32)
            candQ = singles.tile([128, L, 4, 4], F32)
            bq2 = bass.AP(
                tensor=ST[:].tensor, offset=(L - 1) * 32 + 16,
                ap=[[L * 32, 128], [-32, HL], [1, 4], [4, 4]])
            nc.vector.tensor_tensor(
                candQ[:, HL:L],
                bq2,
                tbc.unsqueeze(1).unsqueeze(1).to_broadcast((128, HL, 4, 4)),
                OP.add)
            nc.vector.reduce_max(Q[:, HL:L], candQ[:, HL:L], axis=AX.X)
            # tbmid[y] = max_m(G'_16[y,m] + Q_16[m])  (G'_16 parked at GG[31,1])
            tbmid = singles.tile([128, 4], F32)
            tmtmp = singles.tile([128, 4, 4], F32)
            nc.vector.tensor_tensor(
                tmtmp, GG[:, L - 1, 1],
                Q[:, HL, :].unsqueeze(1).to_broadcast((128, 4, 4)), OP.add)
            nc.vector.reduce_max(tbmid, tmtmp, axis=AX.X)
            # Q for steps 0..15: B1_u[y,j] = ST[15-u, 1, y, j]
            bq1 = bass.AP(
                tensor=ST[:].tensor, offset=(HL - 1) * 32 + 16,
                ap=[[L * 32, 128], [-32, HL], [1, 4], [4, 4]])
            nc.vector.tensor_tensor(
                candQ[:, 0:HL],
                bq1,
                tbmid.unsqueeze(1).unsqueeze(1).to_broadcast((128, HL, 4, 4)),
                OP.add)
            nc.vector.reduce_max(Q[:, 0:HL], candQ[:, 0:HL], axis=AX.X)

            # sbmid[m] = max_i(sb[i] + mid1[i,m])
            sbmid = singles.tile([128, 4], F32)
            smtmp = singles.tile([128, 4, 4], F32)
            nc.vector.tensor_tensor(
                smtmp, mid1.transpose([0, 2, 1]),
                sbc.unsqueeze(1).to_broadcast((128, 4, 4)), OP.add)
            nc.vector.reduce_max(sbmid, smtmp, axis=AX.X)
            # P halves
            P = singles.tile([128, L, 4], F32)
            candP = singles.tile([128, L, 4, 4], F32)
            nc.vector.tensor_tensor(
                candP[:, 0:HL],
                ST[:, 0:HL, 0].transpose([0, 1, 3, 2]),
                sbc.unsqueeze(1).unsqueeze(1).to_broadcast((128, HL, 4, 4)),
                OP.add)
            nc.vector.reduce_max(P[:, 0:HL], candP[:, 0:HL], axis=AX.X)
            nc.vector.tensor_tensor(
                candP[:, HL:L],
                ST[:, HL:L, 0].transpose([0, 1, 3, 2]),
                sbmid.unsqueeze(1).unsqueeze(1).to_broadcast((128, HL, 4, 4)),
                OP.add)
            nc.vector.reduce_max(P[:, HL:L], candP[:, HL:L], axis=AX.X)
            R = singles.tile([128, L, 4], F32)
            nc.vector.tensor_tensor(R, P, emv, OP.add)
            nc.vector.tensor_tensor(R, R, Q, OP.add)
            M = singles.tile([128, L], F32)
            nc.vector.reduce_max(M, R, axis=AX.X)
            eq = singles.tile([128, L, 4], F32)
            nc.vector.tensor_tensor(
                eq, R, M.unsqueeze(2).to_broadcast((128, L, 4)), OP.is_equal)
            nc.vector.tensor_tensor(
                eq, eq, iw4.unsqueeze(1).to_broadcast((128, L, 4)), OP.mult)
            tagf = singles.tile([128, L], F32)
            nc.vector.reduce_max(tagf, eq, axis=AX.X)
            tagi = singles.tile([128, L], I32)
            nc.vector.tensor_copy(tagi, tagf)
            nc.sync.dma_start(tagsd[:].rearrange("b (c t) -> b c t", c=C), tagi)
            sCB.__exit__(None, None, None)

    return nc


def _split_multi_waits(nc):
    """Walrus (bass2jax path) allows very few embedded sync waits per
    instruction (PE matmul: exactly 1). Hoist multi-waits onto standalone
    single-wait InstDrain instructions on the same engine, preserving order."""
    for f in nc.m.functions:
        for blk in f.blocks:
            insts = blk.instructions
            i = 0
            while i < len(insts):
                ins = insts[i]
                si = ins.sync_info
                w = list(si.on_wait) if (si is not None and si.on_wait) else []
                if len(w) >= 2:
                    for k, wait in enumerate(w):
                        d = mybir.InstEventSemaphore(
                            name=nc.get_next_instruction_name(), ins=[], outs=[])
                        d.engine = ins.engine
                        d.sync_info = mybir.SyncInfo(on_wait=[wait], on_update=[])
                        insts.insert(i + k, d)
                    i += len(w)
                    ins.sync_info = mybir.SyncInfo(
                        on_wait=[], on_update=list(si.on_update or []))
                i += 1


def _get_nc():
    if "nc" not in _NC_CACHE:
        nc = build_nc()
        _split_multi_waits(nc)   # HW path only; CoreSim rejects raw drains
        _NC_CACHE["nc"] = nc
    return _NC_CACHE["nc"]


def make_in_maps(inputs):
    sent = np.ascontiguousarray(np.asarray(inputs["sentences"], dtype=np.float32))
    W = np.ascontiguousarray(np.asarray(inputs["W"], dtype=np.float32))
    bb = np.ascontiguousarray(np.asarray(inputs["b"], dtype=np.float32))
    st = np.ascontiguousarray(np.asarray(inputs["start_transitions"], dtype=np.float32))
    en = np.ascontiguousarray(np.asarray(inputs["end_transitions"], dtype=np.float32))
    tr = np.ascontiguousarray(np.asarray(inputs["transitions"], dtype=np.float32))
    tinit = np.tile(tr.ravel(), (128, 1)).astype(np.float32)
    tinit[0::C, :] = np.tile(st, 4)[None, :]
    binit = np.full((4, 4), NEG, dtype=np.float32)
    np.fill_diagonal(binit, 0.0)
    rc = np.zeros((128, 64), dtype=np.float32)
    rc[:, 0:4] = [0.0, 1.0, 2.0, 3.0]
    rc[0:K, 4] = bb
    rc[:, 8:24] = tinit
    rc[:, 24:40] = binit.ravel()[None, :]
    rc[:, 40:56] = tr.ravel()[None, :]
    rc[:, 56:60] = en[None, :]
    rc[0:K, 60:64] = np.eye(K, dtype=np.float32)
    identc = np.eye(128, dtype=np.float32)
    return [{
        "sentences": sent[c * BC:(c + 1) * BC],
        "W": W, "identc": identc, "rowconsts": rc,
    } for c in range(NCORES)]


def kernel(**inputs):
    nc = _get_nc()
    in_maps = make_in_maps(inputs)
    res = run_bass_kernel_spmd(nc, in_maps, core_ids=list(range(NCORES)))
    tags = np.concatenate([res.results[c]["tags"] for c in range(NCORES)], axis=0)
    return tags.astype(np.int32)


if __name__ == "__main__":
    import reference
    inputs = {k: np.asarray(v) for k, v in reference.setup_inputs().items()}
    out = kernel(**inputs)
    print(out.shape, out.dtype, out[:2, :16])
